# revision 14
# baseline (speedup 1.0000x reference)
"""Trainium2 Bass kernel for CapsDecorelationNormalization.

x[B=2048, CI=32, CO=32, A=16] fp32: center over (B, CO) per (CI, A);
per-capsule covariance sigma[CI, A, A]; Newton-Schulz inverse-sqrt (5 iters);
whiten; * gamma + beta.

Sharding: capsule-parallel (CI) across the 8 cores -- 4 capsules per core.
Every core sees ALL B*CO = 65536 samples for its own capsules, so the
covariance is complete locally and NO collective is needed; the cores are
fully independent (no cross-core skew sensitivity).

Per core (D = 4 caps x 16 atoms = 64, N = 65536 samples), all data bf16
(tolerance 2e-2; bf16 end-to-end measures 8.8e-3 in simulation):

  Host marshals two layouts (both bf16):
    xs [128, 256, 129]: chunk-pair j holds samples [256j,256j+256) as
       [even-128-samples cols 0:64 | odd-128 cols 64:128 | ones col 128]
    xt [128, 32768]: rows 0:64 = x^T for samples 0:32768, rows 64:128 =
       x^T for samples 32768:65536 (atoms on partitions, 128-wide packed)
  Phase 1 (cov): 256 matmuls lhsT=xs[:,j,0:128] (128-col bf16 weight ->
    FWL), rhs=xs[:,j,0:129], one PSUM accumulation -> S_even/S_odd blocks
    + per-atom sums in col 128.
  Phase 2 (tiny): fold halves, sigma=(S-N mu mu^T)/(N-1) block-masked,
    per-capsule trace norm, Newton-Schulz x5 on the 64x64 block-diag,
    fold gamma -> W', build WBD2[128,128] = diag(W',W') bf16 and
    bias column = (beta - mu W') replicated to both halves.
  Phase 3 (whiten): 64 matmuls lhsT=WBD2 (stationary), rhs=xt[:,t*512:...]
    -> dec^T tiles; DVE/ACT evacuate PSUM with bias add -> bf16 -> DMA out.
  Output out^T [128, 32768] bf16; host un-marshals and upcasts.
"""

import numpy as np
from contextlib import ExitStack

import ml_dtypes

import concourse.bass as bass
import concourse.tile as tile
from concourse import bacc, mybir
from concourse.masks import make_identity
from concourse.bass_utils import run_bass_kernel_spmd
from concourse.tile import add_dep_helper

B, CI, CO, A = 2048, 32, 32, 16
NCORES = 8
CPC = CI // NCORES          # 4 capsules per core
D = CPC * A                 # 64 (cap,atom) columns
NSAMP = B * CO              # 65536 samples per capsule
NPAIR = NSAMP // 256        # 256 chunk-pairs (2x128 samples each)
PW = 2 * D + 1              # 129 cols per pair (even | odd | ones)
HALF = NSAMP // 2           # 32768
WN = 512                    # whiten tile width (1 PSUM bank fp32)
WT = HALF // WN             # 64 whiten tiles
ITERS = 5
XSPC = 8                    # xs DMA pieces
XTPC = 8                    # xt DMA pieces
OPC = 8                     # out DMA pieces
F32 = mybir.dt.float32
BF16 = mybir.dt.bfloat16
FP8 = mybir.dt.float8e4
BFNP = ml_dtypes.bfloat16
F8NP = ml_dtypes.float8_e4m3

_DRAM = {}


def caps_kernel(ctx, tc):
    nc = tc.nc
    if id(nc) not in _DRAM:
        _DRAM.clear()
        _DRAM[id(nc)] = (
            nc.dram_tensor("xs", [128, NPAIR, PW], FP8, kind="ExternalInput"),
            nc.dram_tensor("xt", [128, HALF], BF16, kind="ExternalInput"),
            nc.dram_tensor("gr", [1, D], F32, kind="ExternalInput"),
            nc.dram_tensor("bt", [1, D], F32, kind="ExternalInput"),
            nc.dram_tensor("outT", [128, HALF], BF16, kind="ExternalOutput"))
    xs, xt, gr, bt, outT = _DRAM[id(nc)]

    singles = ctx.enter_context(tc.tile_pool(name="singles", bufs=1))
    work = ctx.enter_context(tc.tile_pool(name="work", bufs=2))
    oring = ctx.enter_context(tc.tile_pool(name="oring", bufs=2))

    # ---- constants ----
    ident = singles.tile([128, 128], F32, tag="ident", name="ident")
    make_identity(nc, ident)
    i64 = ident[0:64, 0:64]
    # I2stack[p, m] = 1 iff p % 64 == m  (128x64)
    i2s = singles.tile([128, 64], F32, tag="i2s", name="i2s")
    nc.vector.tensor_add(out=i2s, in0=ident[:, 0:64], in1=ident[:, 64:128])
    # J[p, m] = 1 iff m % 64 == p  (64x128)
    jrep = singles.tile([64, 128], F32, tag="jrep", name="jrep")
    nc.vector.tensor_copy(out=jrep[:, 0:64], in_=i64)
    nc.scalar.copy(out=jrep[:, 64:128], in_=i64)
    # capsel_T [4, 64]: 1 iff col // 16 == p
    cselT = singles.tile([4, 64], F32, tag="cselT", name="cselT")
    nc.gpsimd.memset(cselT, 1.0)
    nc.gpsimd.affine_select(out=cselT, in_=cselT,
                            compare_op=mybir.AluOpType.is_ge, fill=0.0,
                            base=0, pattern=[[1, 64]], channel_multiplier=-16)
    nc.gpsimd.affine_select(out=cselT, in_=cselT,
                            compare_op=mybir.AluOpType.is_ge, fill=0.0,
                            base=15, pattern=[[-1, 64]], channel_multiplier=16)
    # capsel [64, 4]: 1 iff col == p // 16
    csel = singles.tile([64, 4], F32, tag="csel", name="csel")
    nc.gpsimd.memset(csel, 1.0)
    nc.gpsimd.affine_select(out=csel, in_=csel,
                            compare_op=mybir.AluOpType.is_ge, fill=0.0,
                            base=0, pattern=[[-16, 4]], channel_multiplier=1)
    nc.gpsimd.affine_select(out=csel, in_=csel,
                            compare_op=mybir.AluOpType.is_ge, fill=0.0,
                            base=15, pattern=[[16, 4]], channel_multiplier=-1)
    ones_row = singles.tile([1, 64], F32, tag="ones_row", name="ones_row")
    nc.vector.memset(ones_row, 1.0)
    # 1.5*I for the Newton-Schulz first-iteration shortcut
    i15 = singles.tile([64, 64], F32, tag="i15", name="i15")
    nc.vector.tensor_scalar_mul(out=i15, in0=ident[0:64, 0:64], scalar1=1.5)

    with tc.tile_pool(name="psc", bufs=1, space="PSUM") as psc:
        bm_ps = psc.tile([64, 64], F32, tag="cps", name="bm_ps")
        nc.tensor.matmul(bm_ps, cselT, cselT, start=True, stop=True)
        bmask = singles.tile([64, 64], F32, tag="bmask", name="bmask")
        nc.scalar.copy(out=bmask, in_=bm_ps)

    # gamma/beta rows + gamma broadcast (no dep on x)
    grow = singles.tile([1, D], F32, tag="grow", name="grow")
    nc.sync.dma_start(out=grow, in_=gr[:, :])
    brow_b = singles.tile([1, D], F32, tag="brow_b", name="brow_b")
    nc.sync.dma_start(out=brow_b, in_=bt[:, :])
    with tc.tile_pool(name="psg", bufs=1, space="PSUM") as psg:
        g_ps = psg.tile([64, 64], F32, tag="gps", name="g_ps")
        nc.tensor.matmul(g_ps, ones_row, grow, start=True, stop=True)
        grep = singles.tile([64, 64], F32, tag="grep", name="grep")
        nc.scalar.copy(out=grep, in_=g_ps)

    # ---- input DMAs (xs first; xt pieces gated behind cov progress so the
    # covariance path gets full HBM bandwidth) ----
    xs_sb = singles.tile([128, NPAIR, PW], FP8, tag="xs_sb", name="xs_sb")
    pj = NPAIR // XSPC
    xs_dmas = []
    for p in range(XSPC):
        xi = nc.sync.dma_start(out=xs_sb[:, p * pj:(p + 1) * pj, :],
                               in_=xs[:, p * pj:(p + 1) * pj, :])
        xs_dmas.append(xi)
    xt_sb = singles.tile([128, WT, WN], BF16, tag="xt_sb", name="xt_sb")

    # ---- phase 1: covariance accumulation ----
    with tc.tile_pool(name="pscov", bufs=1, space="PSUM") as pscov, \
         tc.tile_pool(name="ps2", bufs=2, space="PSUM") as ps2:
        cov_ps = pscov.tile([128, PW], F32, tag="cov", name="cov_ps")
        for j in range(NPAIR):
            nc.tensor.matmul(cov_ps, xs_sb[:, j, 0:128], xs_sb[:, j, :],
                             start=(j == 0), stop=(j == NPAIR - 1))

        # xt streams right after the (smaller, covariance-gating) xs input
        pt = HALF // XTPC
        wt_p = WT // XTPC
        for p in range(XTPC):
            di = nc.sync.dma_start(
                out=xt_sb[:, p * wt_p:(p + 1) * wt_p, :],
                in_=xt[:, p * pt:(p + 1) * pt])
            add_dep_helper(di.ins, xs_dmas[-1].ins, sync=True,
                           reason="give xs DMA priority over xt")

        # ---- phase 2 ----
        sfull = singles.tile([128, PW], F32, tag="sfull", name="sfull")
        nc.vector.tensor_copy(out=sfull, in_=cov_ps)
        # fold odd-half block (partitions 64:128, cols 64:129) down to 0:64
        f_ps = ps2.tile([64, 65], F32, tag="psA", name="f_ps")
        nc.tensor.matmul(f_ps, i2s[64:128, :], sfull[64:128, 64:129],
                         start=True, stop=True)
        stot = singles.tile([64, 65], F32, tag="stot", name="stot")
        nc.vector.tensor_add(out=stot[:, 0:64], in0=sfull[0:64, 0:64],
                             in1=f_ps[:, 0:64])
        nc.vector.tensor_add(out=stot[:, 64:65], in0=sfull[0:64, 128:129],
                             in1=f_ps[:, 64:65])
        # mu and mu row
        mu = singles.tile([64, 1], F32, tag="mu", name="mu")
        nc.vector.tensor_scalar_mul(out=mu, in0=stot[:, 64:65],
                                    scalar1=1.0 / NSAMP)
        mur_ps = ps2.tile([1, 64], F32, tag="psB", name="mur_ps")
        nc.tensor.transpose(mur_ps, mu, i64)
        mur = work.tile([1, 64], F32, tag="mur", name="mur")
        nc.scalar.copy(out=mur, in_=mur_ps)
        outer_ps = ps2.tile([64, 64], F32, tag="psA", name="outer_ps")
        nc.tensor.matmul(outer_ps, mur, mur, start=True, stop=True)
        # sigma = (S - N mu mu^T) / (N-1), block-masked
        inv_nm1 = 1.0 / (NSAMP - 1.0)
        sig = singles.tile([64, 64], F32, tag="sig", name="sig")
        nc.vector.tensor_scalar_mul(out=sig, in0=stot[:, 0:64],
                                    scalar1=inv_nm1)
        osc = work.tile([64, 64], F32, tag="osc", name="osc")
        nc.scalar.activation(out=osc, in_=outer_ps,
                             func=mybir.ActivationFunctionType.Copy,
                             scale=NSAMP * inv_nm1)
        nc.vector.tensor_sub(out=sig, in0=sig, in1=osc)
        nc.vector.tensor_mul(out=sig, in0=sig, in1=bmask)
        # per-capsule traces -> 1/tr and 1/sqrt(tr) columns
        diag = work.tile([64, 64], F32, tag="diag", name="diag")
        nc.vector.tensor_mul(out=diag, in0=sig, in1=i64)
        dcol = work.tile([64, 1], F32, tag="dcol", name="dcol")
        nc.vector.tensor_reduce(out=dcol, in_=diag,
                                axis=mybir.AxisListType.X,
                                op=mybir.AluOpType.add)
        tr4_ps = ps2.tile([1, 4], F32, tag="psB", name="tr4_ps")
        nc.tensor.matmul(tr4_ps, dcol, csel, start=True, stop=True)
        tr4 = work.tile([1, 8], F32, tag="tr4", name="tr4")
        nc.vector.reciprocal(out=tr4[:, 0:4], in_=tr4_ps)
        nc.scalar.activation(out=tr4[:, 4:8], in_=tr4[:, 0:4],
                             func=mybir.ActivationFunctionType.Sqrt)
        c8_ps = ps2.tile([4, 2], F32, tag="psB", name="c8_ps")
        nc.tensor.transpose(c8_ps[:, 0:1], tr4[:, 0:4], ones_row[:, 0:1])
        nc.tensor.transpose(c8_ps[:, 1:2], tr4[:, 4:8], ones_row[:, 0:1])
        c8 = work.tile([4, 2], F32, tag="c8", name="c8")
        nc.scalar.copy(out=c8, in_=c8_ps)
        trc_ps = ps2.tile([64, 2], F32, tag="psB", name="trc_ps")
        nc.tensor.matmul(trc_ps, cselT, c8, start=True, stop=True)
        trcol = singles.tile([64, 2], F32, tag="trcol", name="trcol")
        nc.scalar.copy(out=trcol, in_=trc_ps)
        # Newton-Schulz on the 64x64 block-diagonal
        sn = singles.tile([64, 64], F32, tag="sn", name="sn")
        nc.vector.tensor_scalar_mul(out=sn, in0=sig, scalar1=trcol[:, 0:1])
        # iter 1 with p0 = I collapses to p1 = 1.5 I - 0.5 sn
        pns = singles.tile([64, 64], F32, tag="pns", name="pns")
        snh = work.tile([64, 64], F32, tag="snh", name="snh")
        nc.vector.tensor_scalar_mul(out=snh, in0=sn, scalar1=0.5)
        nc.vector.tensor_sub(out=pns, in0=i15, in1=snh)
        for _ in range(ITERS - 1):
            u_ps = ps2.tile([64, 64], F32, tag="psA", name="u_ps")
            v_ps = ps2.tile([64, 64], F32, tag="psB", name="v_ps")
            nc.tensor.matmul(u_ps, pns, sn, start=True, stop=True)
            nc.tensor.matmul(v_ps, pns, pns, start=True, stop=True)
            uv = work.tile([64, 2, 64], F32, tag="uv", name="uv")
            nc.scalar.copy(out=uv[:, 0, :], in_=u_ps)
            nc.vector.tensor_copy(out=uv[:, 1, :], in_=v_ps)
            t_ps = ps2.tile([64, 64], F32, tag="psA", name="t_ps")
            nc.tensor.matmul(t_ps, uv[:, 1, :], uv[:, 0, :],
                             start=True, stop=True)
            nc.vector.tensor_scalar_mul(out=pns, in0=pns, scalar1=1.5)
            th = work.tile([64, 64], F32, tag="th", name="th")
            nc.scalar.activation(out=th, in_=t_ps,
                                 func=mybir.ActivationFunctionType.Copy,
                                 scale=0.5)
            nc.vector.tensor_sub(out=pns, in0=pns, in1=th)
        # w' = p * rsqrt(tr) * gamma(col)
        wp = singles.tile([64, 64], F32, tag="wp", name="wp")
        nc.vector.tensor_scalar_mul(out=wp, in0=pns, scalar1=trcol[:, 1:2])
        nc.vector.tensor_mul(out=wp, in0=wp, in1=grep)
        # WBD2 = diag(w', w') bf16
        wrep_ps = ps2.tile([128, 64], F32, tag="psA", name="wrep_ps")
        nc.tensor.matmul(wrep_ps, jrep, wp, start=True, stop=True)
        wbd2 = singles.tile([128, 128], BF16, tag="wbd2", name="wbd2")
        nc.vector.memset(wbd2, 0.0)
        nc.vector.tensor_copy(out=wbd2[0:64, 0:64], in_=wrep_ps[0:64, :])
        nc.vector.tensor_copy(out=wbd2[64:128, 64:128],
                              in_=wrep_ps[64:128, :])
        # bias column = (beta - mu @ w') replicated to both halves
        bm2_ps = ps2.tile([1, 64], F32, tag="psB", name="bm2_ps")
        nc.tensor.matmul(bm2_ps, mu, wp, start=True, stop=True)
        brow = work.tile([1, 64], F32, tag="brow", name="brow")
        nc.vector.tensor_sub(out=brow, in0=brow_b, in1=bm2_ps)
        b64_ps = ps2.tile([64, 1], F32, tag="psB", name="b64_ps")
        nc.tensor.transpose(b64_ps, brow, ones_row[:, 0:1])
        b64 = work.tile([64, 1], F32, tag="b64", name="b64")
        nc.scalar.copy(out=b64, in_=b64_ps)
        bc_ps = ps2.tile([128, 1], F32, tag="psA", name="bc_ps")
        nc.tensor.matmul(bc_ps, jrep, b64, start=True, stop=True)
        biascol = singles.tile([128, 1], F32, tag="biascol", name="biascol")
        nc.scalar.copy(out=biascol, in_=bc_ps)

    # ---- phase 3: whiten + bias + store ----
    # two matmuls fill a 2-bank PSUM tile; ONE evacuation op (alternating
    # DVE / ACT) reads both banks, adds bias, casts to bf16
    tpo = WT // OPC
    with tc.tile_pool(name="psdec", bufs=3, space="PSUM") as psdec:
        for p in range(OPC):
            out_sb = oring.tile([128, tpo, WN], BF16, tag="out_sb",
                                name="out_sb")
            for i in range(0, tpo, 2):
                t = p * tpo + i
                dec_ps = psdec.tile([128, 2, WN], F32, tag="dec",
                                    name="dec_ps")
                nc.tensor.matmul(dec_ps[:, 0, :], wbd2, xt_sb[:, t, :],
                                 start=True, stop=True)
                nc.tensor.matmul(dec_ps[:, 1, :], wbd2, xt_sb[:, t + 1, :],
                                 start=True, stop=True)
                dst = out_sb[:, i:i + 2, :].rearrange("p a b -> p (a b)")
                src = dec_ps.rearrange("p a b -> p (a b)")
                if (i // 2) % 2 == 0:
                    nc.vector.tensor_scalar_add(out=dst, in0=src,
                                                scalar1=biascol)
                else:
                    nc.scalar.add(out=dst, in_=src, add=biascol)
            nc.sync.dma_start(
                out=outT[:, p * tpo * WN:(p + 1) * tpo * WN],
                in_=out_sb)


_NC_CACHE = {}


def build_nc(repeat=1):
    key = f"nc{repeat}"
    if key not in _NC_CACHE:
        nc = bacc.Bacc(None, num_devices=NCORES)
        with ExitStack() as ctx:
            tc = ctx.enter_context(tile.TileContext(nc))
            for _ in range(repeat):
                caps_kernel(ctx, tc)
        nc.finalize()
        _NC_CACHE[key] = nc
    return _NC_CACHE[key]


def make_in_maps(inputs):
    x = np.asarray(inputs["x"], dtype=np.float32)
    gamma = np.asarray(inputs["gamma"], dtype=np.float32)
    beta = np.asarray(inputs["beta"], dtype=np.float32)
    in_maps = []
    for i in range(NCORES):
        caps = slice(i * CPC, (i + 1) * CPC)
        xflat = np.ascontiguousarray(
            x[:, caps].transpose(0, 2, 1, 3)).reshape(NSAMP, D)
        xq = xflat.astype(BFNP)
        x8 = xflat.astype(F8NP)
        xs_host = np.empty((128, NPAIR, PW), dtype=F8NP)
        tmp = x8.reshape(NPAIR, 2, 128, D)
        xs_host[:, :, 0:D] = tmp[:, 0].transpose(1, 0, 2)
        xs_host[:, :, D:2 * D] = tmp[:, 1].transpose(1, 0, 2)
        xs_host[:, :, 2 * D] = F8NP(1.0)
        xt_host = np.empty((128, HALF), dtype=BFNP)
        xt_host[0:D] = xq[:HALF].T
        xt_host[D:2 * D] = xq[HALF:].T
        in_maps.append({
            "xs": xs_host,
            "xt": xt_host,
            "gr": np.ascontiguousarray(
                gamma[0, caps, 0, :].reshape(1, D)),
            "bt": np.ascontiguousarray(
                beta[0, caps, 0, :].reshape(1, D)),
        })
    return in_maps


def kernel(x, gamma, beta):
    nc = build_nc()
    in_maps = make_in_maps({"x": x, "gamma": gamma, "beta": beta})
    res = run_bass_kernel_spmd(nc, in_maps, list(range(NCORES)))
    out = np.empty((B, CI, CO, A), dtype=np.float32)
    for i in range(NCORES):
        caps = slice(i * CPC, (i + 1) * CPC)
        ot = np.asarray(res.results[i]["outT"])
        decflat = np.concatenate(
            [ot[0:D].T, ot[D:2 * D].T], axis=0).astype(np.float32)
        out[:, caps] = decflat.reshape(B, CO, CPC, A).transpose(0, 2, 1, 3)
    return out


# revision 28
# speedup vs baseline: 1.1342x; 1.1342x over previous
"""Trainium2 Bass kernel for CapsDecorelationNormalization.

x[B=2048, CI=32, CO=32, A=16] fp32: center over (B, CO) per (CI, A);
per-capsule covariance sigma[CI, A, A]; Newton-Schulz inverse-sqrt (5 iters);
whiten; * gamma + beta.

Sharding: capsule-parallel (CI) across the 8 cores -- 4 capsules per core.
Every core sees ALL B*CO = 65536 samples for its own capsules, so the
covariance is complete locally and NO collective is needed; the cores are
fully independent (no cross-core skew sensitivity).

Per core (D = 4 caps x 16 atoms = 64, N = 65536 samples), all data bf16
(tolerance 2e-2; bf16 end-to-end measures 8.8e-3 in simulation):

  Host marshals two layouts (both bf16):
    xs [128, 256, 129]: chunk-pair j holds samples [256j,256j+256) as
       [even-128-samples cols 0:64 | odd-128 cols 64:128 | ones col 128]
    xt [128, 32768]: rows 0:64 = x^T for samples 0:32768, rows 64:128 =
       x^T for samples 32768:65536 (atoms on partitions, 128-wide packed)
  Phase 1 (cov): 256 matmuls lhsT=xs[:,j,0:128] (128-col bf16 weight ->
    FWL), rhs=xs[:,j,0:129], one PSUM accumulation -> S_even/S_odd blocks
    + per-atom sums in col 128.
  Phase 2 (tiny): fold halves, sigma=(S-N mu mu^T)/(N-1) block-masked,
    per-capsule trace norm, Newton-Schulz x5 on the 64x64 block-diag,
    fold gamma -> W', build WBD2[128,128] = diag(W',W') bf16 and
    bias column = (beta - mu W') replicated to both halves.
  Phase 3 (whiten): 64 matmuls lhsT=WBD2 (stationary), rhs=xt[:,t*512:...]
    -> dec^T tiles; DVE/ACT evacuate PSUM with bias add -> bf16 -> DMA out.
  Output out^T [128, 32768] bf16; host un-marshals and upcasts.
"""

import numpy as np
from contextlib import ExitStack

import ml_dtypes

import concourse.bass as bass
import concourse.tile as tile
from concourse import bacc, mybir
from concourse.masks import make_identity
from concourse.bass_utils import run_bass_kernel_spmd
from concourse.tile import add_dep_helper

B, CI, CO, A = 2048, 32, 32, 16
NCORES = 8
CPC = CI // NCORES          # 4 capsules per core
D = CPC * A                 # 64 (cap,atom) columns
NSAMP = B * CO              # 65536 samples per capsule
NPAIR = NSAMP // 256        # 256 chunk-pairs (2x128 samples each)
PW = 2 * D + 1              # 129 cols per pair (even | odd | ones)
HALF = NSAMP // 2           # 32768
WN = 512                    # whiten tile width (1 PSUM bank fp32)
WT = HALF // WN             # 64 whiten tiles
ITERS = 5
XSPC = 8                    # xs DMA pieces
XTPC = 8                    # xt DMA pieces
OPC = 8                     # out DMA pieces
F32 = mybir.dt.float32
F32R = mybir.dt.float32r
BF16 = mybir.dt.bfloat16
FP8 = mybir.dt.float8e4
BFNP = ml_dtypes.bfloat16
F8NP = ml_dtypes.float8_e4m3

_DRAM = {}


def caps_kernel(ctx, tc):
    nc = tc.nc
    if id(nc) not in _DRAM:
        _DRAM.clear()
        _DRAM[id(nc)] = (
            nc.dram_tensor("xs", [128, NPAIR, PW], FP8, kind="ExternalInput"),
            nc.dram_tensor("xt", [128, HALF], BF16, kind="ExternalInput"),
            nc.dram_tensor("gr", [1, D], F32, kind="ExternalInput"),
            nc.dram_tensor("bt", [1, D], F32, kind="ExternalInput"),
            nc.dram_tensor("outT", [128, HALF], BF16, kind="ExternalOutput"))
    xs, xt, gr, bt, outT = _DRAM[id(nc)]

    singles = ctx.enter_context(tc.tile_pool(name="singles", bufs=1))
    work = ctx.enter_context(tc.tile_pool(name="work", bufs=2))
    oring = ctx.enter_context(tc.tile_pool(name="oring", bufs=4))

    # ---- constants ----
    ident = singles.tile([128, 128], F32, tag="ident", name="ident")
    make_identity(nc, ident)
    i64 = ident[0:64, 0:64]
    i64r = singles.tile([64, 64], F32R, tag="i64r", name="i64r")
    nc.vector.tensor_copy(out=i64r, in_=i64)
    # I2stack[p, m] = 1 iff p % 64 == m  (128x64)
    i2s = singles.tile([128, 64], F32, tag="i2s", name="i2s")
    nc.vector.tensor_add(out=i2s, in0=ident[:, 0:64], in1=ident[:, 64:128])
    # J[p, m] = 1 iff m % 64 == p  (64x128)
    jrep = singles.tile([64, 128], F32R, tag="jrep", name="jrep")
    nc.vector.tensor_copy(out=jrep[:, 0:64], in_=i64)
    nc.scalar.copy(out=jrep[:, 64:128], in_=i64)
    jrepf = singles.tile([64, 128], F32, tag="jrepf", name="jrepf")
    nc.vector.tensor_copy(out=jrepf, in_=jrep)
    # capsel_T [4, 64]: 1 iff col // 16 == p
    cselT = singles.tile([4, 64], F32, tag="cselT", name="cselT")
    nc.gpsimd.memset(cselT, 1.0)
    nc.gpsimd.affine_select(out=cselT, in_=cselT,
                            compare_op=mybir.AluOpType.is_ge, fill=0.0,
                            base=0, pattern=[[1, 64]], channel_multiplier=-16)
    nc.gpsimd.affine_select(out=cselT, in_=cselT,
                            compare_op=mybir.AluOpType.is_ge, fill=0.0,
                            base=15, pattern=[[-1, 64]], channel_multiplier=16)
    # capsel [64, 4]: 1 iff col == p // 16
    csel = singles.tile([64, 4], F32, tag="csel", name="csel")
    nc.gpsimd.memset(csel, 1.0)
    nc.gpsimd.affine_select(out=csel, in_=csel,
                            compare_op=mybir.AluOpType.is_ge, fill=0.0,
                            base=0, pattern=[[-16, 4]], channel_multiplier=1)
    nc.gpsimd.affine_select(out=csel, in_=csel,
                            compare_op=mybir.AluOpType.is_ge, fill=0.0,
                            base=15, pattern=[[16, 4]], channel_multiplier=-1)
    ones_row = singles.tile([1, 64], F32, tag="ones_row", name="ones_row")
    nc.vector.memset(ones_row, 1.0)
    # 1.5*I for the Newton-Schulz first-iteration shortcut
    i15 = singles.tile([64, 64], F32, tag="i15", name="i15")
    nc.vector.tensor_scalar_mul(out=i15, in0=ident[0:64, 0:64], scalar1=1.5)
    # preload the ACT Sqrt table so it is not loaded mid phase-2
    sqdum = singles.tile([1, 1], F32, tag="sqdum", name="sqdum")
    nc.scalar.activation(out=sqdum, in_=ones_row[:, 0:1],
                         func=mybir.ActivationFunctionType.Sqrt)

    with tc.tile_pool(name="psc", bufs=1, space="PSUM") as psc:
        bm_ps = psc.tile([64, 64], F32, tag="cps", name="bm_ps")
        nc.tensor.matmul(bm_ps, cselT, cselT, start=True, stop=True)
        bmask = singles.tile([64, 64], F32, tag="bmask", name="bmask")
        nc.scalar.copy(out=bmask, in_=bm_ps)

    # gamma/beta rows + gamma broadcast (no dep on x)
    grow = singles.tile([1, D], F32, tag="grow", name="grow")
    nc.sync.dma_start(out=grow, in_=gr[:, :])
    brow_b = singles.tile([1, D], F32, tag="brow_b", name="brow_b")
    nc.sync.dma_start(out=brow_b, in_=bt[:, :])
    with tc.tile_pool(name="psg", bufs=1, space="PSUM") as psg:
        g_ps = psg.tile([64, 64], F32, tag="gps", name="g_ps")
        nc.tensor.matmul(g_ps, ones_row, grow, start=True, stop=True)
        grep = singles.tile([64, 64], F32, tag="grep", name="grep")
        nc.scalar.copy(out=grep, in_=g_ps)

    # ---- input DMAs (xs first; xt pieces gated behind cov progress so the
    # covariance path gets full HBM bandwidth) ----
    xs_sb = singles.tile([128, NPAIR, PW], FP8, tag="xs_sb", name="xs_sb")
    pj = NPAIR // XSPC
    xs_dmas = []
    for p in range(XSPC):
        xi = nc.sync.dma_start(out=xs_sb[:, p * pj:(p + 1) * pj, :],
                               in_=xs[:, p * pj:(p + 1) * pj, :])
        xs_dmas.append(xi)
    xt_sb = singles.tile([128, WT, WN], BF16, tag="xt_sb", name="xt_sb")

    # ---- phase 1: covariance accumulation ----
    with tc.tile_pool(name="pscov", bufs=1, space="PSUM") as pscov, \
         tc.tile_pool(name="ps2", bufs=2, space="PSUM") as ps2:
        cov_ps = pscov.tile([128, PW], F32, tag="cov", name="cov_ps")
        for j in range(NPAIR):
            nc.tensor.matmul(cov_ps, xs_sb[:, j, 0:128], xs_sb[:, j, :],
                             start=(j == 0), stop=(j == NPAIR - 1))

        # xt streams right after the (smaller, covariance-gating) xs input
        pt = HALF // XTPC
        wt_p = WT // XTPC
        for p in range(XTPC):
            di = nc.sync.dma_start(
                out=xt_sb[:, p * wt_p:(p + 1) * wt_p, :],
                in_=xt[:, p * pt:(p + 1) * pt])
            add_dep_helper(di.ins, xs_dmas[-1].ins, sync=True,
                           reason="give xs DMA priority over xt")

        # ---- phase 2 (matmuls in f32r: single-pass on PE vs fp32's
        # LOW/HIGH instruction pair; TF32-class precision is ample) ----
        sfull = singles.tile([128, PW], F32, tag="sfull", name="sfull")
        nc.vector.tensor_copy(out=sfull, in_=cov_ps)
        # fold odd-half block (partitions 64:128, cols 64:129) down to 0:64
        f_ps = ps2.tile([64, 65], F32, tag="psA", name="f_ps")
        nc.tensor.matmul(f_ps, i2s[64:128, :], sfull[64:128, 64:129],
                         start=True, stop=True)
        stot = singles.tile([64, 65], F32, tag="stot", name="stot")
        nc.vector.tensor_add(out=stot[:, 0:64], in0=sfull[0:64, 0:64],
                             in1=f_ps[:, 0:64])
        nc.vector.tensor_add(out=stot[:, 64:65], in0=sfull[0:64, 128:129],
                             in1=f_ps[:, 64:65])
        # mu and mu row
        mu = singles.tile([64, 1], F32R, tag="mu", name="mu")
        nc.vector.tensor_scalar_mul(out=mu, in0=stot[:, 64:65],
                                    scalar1=1.0 / NSAMP)
        mur_ps = ps2.tile([1, 64], F32R, tag="psB", name="mur_ps")
        nc.tensor.transpose(mur_ps, mu, i64r)
        mur = work.tile([1, 64], F32R, tag="mur", name="mur")
        nc.scalar.copy(out=mur, in_=mur_ps)
        outer_ps = ps2.tile([64, 64], F32, tag="psA", name="outer_ps")
        nc.tensor.matmul(outer_ps, mur, mur, start=True, stop=True)
        # sigma = (S - N mu mu^T) / (N-1), block-masked
        inv_nm1 = 1.0 / (NSAMP - 1.0)
        sig = singles.tile([64, 64], F32, tag="sig", name="sig")
        nc.vector.tensor_scalar_mul(out=sig, in0=stot[:, 0:64],
                                    scalar1=inv_nm1)
        osc = work.tile([64, 64], F32, tag="osc", name="osc")
        nc.scalar.activation(out=osc, in_=outer_ps,
                             func=mybir.ActivationFunctionType.Copy,
                             scale=NSAMP * inv_nm1)
        nc.vector.tensor_sub(out=sig, in0=sig, in1=osc)
        nc.vector.tensor_mul(out=sig, in0=sig, in1=bmask)
        # per-capsule traces -> 1/tr and 1/sqrt(tr) columns
        diag = work.tile([64, 64], F32, tag="diag", name="diag")
        nc.vector.tensor_mul(out=diag, in0=sig, in1=i64)
        dcol = work.tile([64, 1], F32, tag="dcol", name="dcol")
        nc.vector.tensor_reduce(out=dcol, in_=diag,
                                axis=mybir.AxisListType.X,
                                op=mybir.AluOpType.add)
        tr4_ps = ps2.tile([1, 4], F32, tag="psB", name="tr4_ps")
        nc.tensor.matmul(tr4_ps, dcol, csel, start=True, stop=True)
        tr4 = work.tile([1, 8], F32, tag="tr4", name="tr4")
        nc.vector.reciprocal(out=tr4[:, 0:4], in_=tr4_ps)
        nc.scalar.activation(out=tr4[:, 4:8], in_=tr4[:, 0:4],
                             func=mybir.ActivationFunctionType.Sqrt)
        c8_ps = ps2.tile([4, 2], F32, tag="psB", name="c8_ps")
        nc.tensor.transpose(c8_ps[:, 0:1], tr4[:, 0:4], ones_row[:, 0:1])
        nc.tensor.transpose(c8_ps[:, 1:2], tr4[:, 4:8], ones_row[:, 0:1])
        c8 = work.tile([4, 2], F32, tag="c8", name="c8")
        nc.scalar.copy(out=c8, in_=c8_ps)
        trc_ps = ps2.tile([64, 2], F32, tag="psB", name="trc_ps")
        nc.tensor.matmul(trc_ps, cselT, c8, start=True, stop=True)
        trcol = singles.tile([64, 2], F32, tag="trcol", name="trcol")
        nc.scalar.copy(out=trcol, in_=trc_ps)
        # Newton-Schulz on the 64x64 block-diagonal
        sn = singles.tile([64, 64], F32R, tag="sn", name="sn")
        nc.vector.tensor_scalar_mul(out=sn, in0=sig, scalar1=trcol[:, 0:1])
        # iter 1 with p0 = I collapses to p1 = 1.5 I - 0.5 sn
        pns = singles.tile([64, 64], F32R, tag="pns", name="pns")
        snh = work.tile([64, 64], F32, tag="snh", name="snh")
        nc.vector.tensor_scalar_mul(out=snh, in0=sn, scalar1=0.5)
        nc.vector.tensor_sub(out=pns, in0=i15, in1=snh)
        for _ in range(ITERS - 1):
            uv_ps = ps2.tile([64, 2, 64], F32, tag="psA", name="uv_ps")
            nc.tensor.matmul(uv_ps[:, 0, :], pns, sn, start=True, stop=True)
            nc.tensor.matmul(uv_ps[:, 1, :], pns, pns, start=True, stop=True)
            uv = work.tile([64, 2, 64], F32R, tag="uv", name="uv")
            nc.vector.tensor_copy(out=uv, in_=uv_ps)
            t_ps = ps2.tile([64, 64], F32, tag="psB", name="t_ps")
            nc.tensor.matmul(t_ps, uv[:, 1, :], uv[:, 0, :],
                             start=True, stop=True)
            nc.vector.tensor_scalar_mul(out=pns, in0=pns, scalar1=1.5)
            th = work.tile([64, 64], F32, tag="th", name="th")
            nc.scalar.activation(out=th, in_=t_ps,
                                 func=mybir.ActivationFunctionType.Copy,
                                 scale=0.5)
            nc.vector.tensor_sub(out=pns, in0=pns, in1=th)
        # w' = p * rsqrt(tr) * gamma(col)
        wp = singles.tile([64, 64], F32R, tag="wp", name="wp")
        nc.vector.tensor_scalar_mul(out=wp, in0=pns, scalar1=trcol[:, 1:2])
        nc.vector.tensor_mul(out=wp, in0=wp, in1=grep)
        # WBD2 = diag(w', w') bf16
        wrep_ps = ps2.tile([128, 64], F32, tag="psA", name="wrep_ps")
        nc.tensor.matmul(wrep_ps, jrep, wp, start=True, stop=True)
        wbd2 = singles.tile([128, 128], BF16, tag="wbd2", name="wbd2")
        nc.vector.memset(wbd2, 0.0)
        nc.vector.tensor_copy(out=wbd2[0:64, 0:64], in_=wrep_ps[0:64, :])
        nc.vector.tensor_copy(out=wbd2[64:128, 64:128],
                              in_=wrep_ps[64:128, :])
        # bias column = (beta - mu @ w') replicated to both halves
        bm2_ps = ps2.tile([1, 64], F32, tag="psB", name="bm2_ps")
        nc.tensor.matmul(bm2_ps, mu, wp, start=True, stop=True)
        brow = work.tile([1, 64], F32, tag="brow", name="brow")
        nc.vector.tensor_sub(out=brow, in0=brow_b, in1=bm2_ps)
        b64_ps = ps2.tile([64, 1], F32, tag="psB", name="b64_ps")
        nc.tensor.transpose(b64_ps, brow, ones_row[:, 0:1])
        b64 = work.tile([64, 1], F32, tag="b64", name="b64")
        nc.scalar.copy(out=b64, in_=b64_ps)
        bc_ps = ps2.tile([128, 1], F32, tag="psA", name="bc_ps")
        nc.tensor.matmul(bc_ps, jrepf, b64, start=True, stop=True)
        biascol = singles.tile([128, 1], F32, tag="biascol", name="biascol")
        nc.scalar.copy(out=biascol, in_=bc_ps)

    # ---- phase 3: whiten + bias + store ----
    # two matmuls fill a 2-bank PSUM tile; ONE evacuation op (alternating
    # DVE / ACT) reads both banks, adds bias, casts to bf16
    tpo = WT // OPC
    with tc.tile_pool(name="psdec", bufs=4, space="PSUM") as psdec:
        for p in range(OPC):
            out_sb = oring.tile([128, tpo, WN], BF16, tag="out_sb",
                                name="out_sb")
            for i in range(0, tpo, 2):
                t = p * tpo + i
                dec_ps = psdec.tile([128, 2, WN], F32, tag="dec",
                                    name="dec_ps")
                nc.tensor.matmul(dec_ps[:, 0, :], wbd2, xt_sb[:, t, :],
                                 start=True, stop=True)
                nc.tensor.matmul(dec_ps[:, 1, :], wbd2, xt_sb[:, t + 1, :],
                                 start=True, stop=True)
                dst = out_sb[:, i:i + 2, :].rearrange("p a b -> p (a b)")
                src = dec_ps.rearrange("p a b -> p (a b)")
                if (i // 2) % 2 == 0:
                    nc.vector.tensor_scalar_add(out=dst, in0=src,
                                                scalar1=biascol)
                else:
                    nc.scalar.add(out=dst, in_=src, add=biascol)
            nc.sync.dma_start(
                out=outT[:, p * tpo * WN:(p + 1) * tpo * WN],
                in_=out_sb)


_NC_CACHE = {}


def build_nc(repeat=1):
    key = f"nc{repeat}"
    if key not in _NC_CACHE:
        nc = bacc.Bacc(None, num_devices=NCORES)
        with ExitStack() as ctx:
            tc = ctx.enter_context(tile.TileContext(nc))
            for _ in range(repeat):
                caps_kernel(ctx, tc)
        nc.finalize()
        _NC_CACHE[key] = nc
    return _NC_CACHE[key]


def make_in_maps(inputs):
    x = np.asarray(inputs["x"], dtype=np.float32)
    gamma = np.asarray(inputs["gamma"], dtype=np.float32)
    beta = np.asarray(inputs["beta"], dtype=np.float32)
    in_maps = []
    for i in range(NCORES):
        caps = slice(i * CPC, (i + 1) * CPC)
        xflat = np.ascontiguousarray(
            x[:, caps].transpose(0, 2, 1, 3)).reshape(NSAMP, D)
        xq = xflat.astype(BFNP)
        x8 = xflat.astype(F8NP)
        xs_host = np.empty((128, NPAIR, PW), dtype=F8NP)
        tmp = x8.reshape(NPAIR, 2, 128, D)
        xs_host[:, :, 0:D] = tmp[:, 0].transpose(1, 0, 2)
        xs_host[:, :, D:2 * D] = tmp[:, 1].transpose(1, 0, 2)
        xs_host[:, :, 2 * D] = F8NP(1.0)
        xt_host = np.empty((128, HALF), dtype=BFNP)
        xt_host[0:D] = xq[:HALF].T
        xt_host[D:2 * D] = xq[HALF:].T
        in_maps.append({
            "xs": xs_host,
            "xt": xt_host,
            "gr": np.ascontiguousarray(
                gamma[0, caps, 0, :].reshape(1, D)),
            "bt": np.ascontiguousarray(
                beta[0, caps, 0, :].reshape(1, D)),
        })
    return in_maps


def kernel(x, gamma, beta):
    nc = build_nc()
    in_maps = make_in_maps({"x": x, "gamma": gamma, "beta": beta})
    res = run_bass_kernel_spmd(nc, in_maps, list(range(NCORES)))
    out = np.empty((B, CI, CO, A), dtype=np.float32)
    for i in range(NCORES):
        caps = slice(i * CPC, (i + 1) * CPC)
        ot = np.asarray(res.results[i]["outT"])
        decflat = np.concatenate(
            [ot[0:D].T, ot[D:2 * D].T], axis=0).astype(np.float32)
        out[:, caps] = decflat.reshape(B, CO, CPC, A).transpose(0, 2, 1, 3)
    return out
